# revision 17
# baseline (speedup 1.0000x reference)
"""Trainium2 Bass kernel for nn_Model_29188597743627 (gnn_message_passing).

Computation (see reference):
  - mention embeddings from bert_emb via start/end gathers, width emb, and
    masked head attention over a <=31-token window per mention
  - mention score MLP + width score
  - pairwise fast antecedent scores fast = src @ me^T + ms_i + ms_j,
    causal mask (j < i) + distance-bucket score, then per-row top-50.

Sharding: 2048 mentions in 16 blocks of 128; core c owns blocks (c, 15-c)
so that causal scan widths are balanced (block k scans j < 128(k+1)).
Every core runs the same SPMD program: block-role A scans a fixed 1024
columns, role B scans 2048 columns; extra columns are killed by the
additive -inf mask template so the structure is core-uniform.

meT (transposed mention embeddings, padded to 2432 rows) is AllGathered
across cores; per-row top-50 is computed with vector.max8/match_replace.
"""

import numpy as np

import concourse.bacc as bacc
import concourse.bass as bass
import concourse.mybir as mybir
import concourse.tile as tile
from concourse.bass import IndirectOffsetOnAxis
from concourse.bass_utils import run_bass_kernel_spmd

P = 128
L = 16384
B = 768
M = 2048
H = 1000
FEAT = 20
MAXW = 30
C = 50
NCORES = 8

# padded mention-embedding layout (d index):
#   [0:768) start | [768:1536) end | [1536:1556) width | pad | 1662 ms/ones |
#   1663 ones/ms | [1664:2432) head
EP = 2432
KT = EP // P  # 19 k-tiles
D_START = 0
D_END = 768
D_WIDTH = 1536
D_HEAD = 1664
E_RAW = 3 * B + FEAT  # 2324

WA_COLS = 1024  # role-A scan width (blocks 0..7 need <=1024)
WB_COLS = 2048  # role-B scan width (blocks 8..15 need <=2048)
NCHUNK_A = WA_COLS // 512  # 2
NCHUNK_B = WB_COLS // 512  # 4

BIN_WIDTHS = [1, 1, 1, 1, 1, 3, 8, 16, 32, 1]
NBINS = len(BIN_WIDTHS)
TTR_LEN = 4096  # reversed template: y<2047 -> ds[bin(2047-y)], else -inf

F32 = mybir.dt.float32
F32R = mybir.dt.float32r
I32 = mybir.dt.int32

NEG_SENTINEL = -3.0e38  # JSON cannot carry -inf in instruction fields


def _r(ap):
    """bitcast an fp32 AP to float32r for full-rate PE matmuls."""
    return ap.bitcast(F32R)


def build_program():
    nc = bacc.Bacc(
        "TRN2", target_bir_lowering=False, debug=False, num_devices=NCORES
    )

    def inp(name, shape, dtype=F32):
        return nc.dram_tensor(name, shape, dtype, kind="ExternalInput")

    # ---- per-core inputs
    bert_emb = inp("bert_emb", [L, B])          # replicated (gather source)
    bert_slice = inp("bert_slice", [M, B])      # rows [2048c, 2048c+2048)
    startsA = inp("startsA", [P, 1], I32)
    startsB = inp("startsB", [P, 1], I32)
    endsA = inp("endsA", [P, 1], I32)
    endsB = inp("endsB", [P, 1], I32)
    widthsA = inp("widthsA", [P, 1], I32)
    widthsB = inp("widthsB", [P, 1], I32)
    widthsfA = inp("widthsfA", [P, 1])
    widthsfB = inp("widthsfB", [P, 1])
    toffA = inp("toffA", [P, 1], I32)           # 2047 - 128*blkA - p
    toffB = inp("toffB", [P, 1], I32)
    # ---- replicated weights / constants
    w_fast_p = inp("w_fast_p", [EP, EP])        # row+col padded W_fast
    bfast_c = inp("bfast_c", [EP, 1])
    wm0_p = inp("wm0_p", [EP, 1024])            # row-padded, col-padded Wm0
    bm0_c = inp("bm0_c", [1024, 1])
    wm1_c = inp("wm1_c", [1024, 1])
    bm1_col = inp("bm1_col", [P, 1])
    ww0 = inp("ww0", [FEAT, 1024])              # col-padded Ww0
    bw0_c = inp("bw0_c", [1024, 1])
    ww1_c = inp("ww1_c", [1024, 1])
    bw1_col = inp("bw1_col", [P, 1])
    wsembT = inp("wsembT", [FEAT, MAXW])        # width_scorer_emb^T
    width_emb = inp("width_emb", [MAXW, FEAT])
    wspan_rep = inp("wspan_rep", [P, B])        # W_span broadcast to 128 rows
    bspan_col = inp("bspan_col", [P, 1])
    dsT = inp("dsT", [FEAT, NBINS])             # dist_scorer_emb^T
    wd = inp("wd", [FEAT, 1])
    bd_col = inp("bd_col", [P, 1])
    onehot_o = inp("onehot_o", [NBINS, TTR_LEN])
    minf_row = inp("minf_row", [1, TTR_LEN])
    id128 = inp("id128", [P, P])
    ninf_col = inp("ninf_col", [P, 1])
    jrep = inp("jrep", [P, MAXW + 1])           # iota 0..30 per row

    outA = nc.dram_tensor("outA", [P, C], F32, kind="ExternalOutput")
    outB = nc.dram_tensor("outB", [P, C], F32, kind="ExternalOutput")

    # ---- internal DRAM
    ttr_dram = nc.dram_tensor("ttr_dram", [TTR_LEN, 1], F32)
    ws_dram = nc.dram_tensor("ws_dram", [MAXW, 1], F32)
    ewa_mine = nc.dram_tensor("ewa_mine", [M, 1], F32)
    ewa_gath = nc.dram_tensor("ewa_gath", [L, 1], F32, addr_space="Shared")
    meT_bounce = nc.dram_tensor("meT_bounce", [EP, 2 * P], F32R)
    meT_gath = nc.dram_tensor(
        "meT_gath", [NCORES * EP, 2 * P], F32R, addr_space="Shared"
    )

    with tile.TileContext(nc) as tc:
        _emit(
            nc, tc,
            bert_emb=bert_emb, bert_slice=bert_slice,
            startsA=startsA, startsB=startsB, endsA=endsA, endsB=endsB,
            widthsA=widthsA, widthsB=widthsB,
            widthsfA=widthsfA, widthsfB=widthsfB,
            toffA=toffA, toffB=toffB,
            w_fast_p=w_fast_p, bfast_c=bfast_c,
            wm0_p=wm0_p, bm0_c=bm0_c, wm1_c=wm1_c, bm1_col=bm1_col,
            ww0=ww0, bw0_c=bw0_c, ww1_c=ww1_c, bw1_col=bw1_col,
            wsembT=wsembT, width_emb=width_emb,
            wspan_rep=wspan_rep, bspan_col=bspan_col,
            dsT=dsT, wd=wd, bd_col=bd_col,
            onehot_o=onehot_o, minf_row=minf_row,
            id128=id128, jrep=jrep, ninf_col=ninf_col,
            outA=outA, outB=outB,
            ttr_dram=ttr_dram, ws_dram=ws_dram,
            ewa_mine=ewa_mine, ewa_gath=ewa_gath,
            meT_bounce=meT_bounce, meT_gath=meT_gath,
        )
    nc.compile()
    return nc


def _emit(nc, tc, **t):
    import os
    stages = int(os.environ.get("KERNEL_STAGES", "8"))
    groups = [list(range(NCORES))]

    ctx_const = tc.tile_pool(name="const", bufs=1)
    ctx_main = tc.tile_pool(name="main", bufs=1)
    const = ctx_const.__enter__()
    main = ctx_main.__enter__()

    # ---------- small constants into SBUF ----------
    def load_const(name, shape, dtype=F32):
        tl = const.tile(shape, dtype, tag=name)
        nc.sync.dma_start(out=tl[:], in_=t[name][:])
        return tl

    id128_sb = load_const("id128", [P, P])
    id128r_sb = const.tile([P, P], F32R, tag="id128r")
    nc.gpsimd.dma_start(out=id128r_sb[:], in_=t["id128"][:, :])
    jrep_sb = load_const("jrep", [P, MAXW + 1])
    wspan_sb = load_const("wspan_rep", [P, B])
    bspan_sb = load_const("bspan_col", [P, 1])
    dsT_sb = load_const("dsT", [FEAT, NBINS])
    wd_sb = load_const("wd", [FEAT, 1])
    bd_sb = load_const("bd_col", [P, 1])
    wsembT_sb = load_const("wsembT", [FEAT, MAXW])
    ww0_sb = load_const("ww0", [FEAT, 1024])
    bm1_sb = load_const("bm1_col", [P, 1])
    ninf_sb = load_const("ninf_col", [P, 1])
    bw1_sb = load_const("bw1_col", [P, 1])

    # [1024,1]-shaped columns viewed as [128, 8] (col h = rows 128h..)
    def load_cols8(name):
        tl = const.tile([P, 8], F32, tag=name)
        nc.sync.dma_start(
            out=tl[:],
            in_=t[name].ap().rearrange("(h p) o -> p (h o)", p=P),
        )
        return tl

    bm0_sb = load_cols8("bm0_c")
    wm1_sb = load_cols8("wm1_c")
    bw0_sb = load_cols8("bw0_c")
    ww1_sb = load_cols8("ww1_c")
    bfast_sb = const.tile([P, KT], F32, tag="bfast")
    nc.sync.dma_start(
        out=bfast_sb[:],
        in_=t["bfast_c"].ap().rearrange("(k p) o -> p (k o)", p=P),
    )

    idx = {}
    for nm in ("startsA", "startsB", "endsA", "endsB", "widthsA", "widthsB",
               "toffA", "toffB"):
        idx[nm] = load_const(nm, [P, 1], I32)
    for nm in ("widthsfA", "widthsfB"):
        idx[nm] = load_const(nm, [P, 1])

    # ---------- distance template ttr ----------
    with tc.tile_pool(name="ttiny", bufs=2, space="PSUM") as ptiny, \
         tc.tile_pool(name="stiny", bufs=2) as stiny:
        ds_ps = ptiny.tile([NBINS, 1], F32, tag="dsps")
        nc.tensor.matmul(ds_ps[:], lhsT=dsT_sb[:], rhs=wd_sb[:],
                         start=True, stop=True)
        ds_sb = stiny.tile([NBINS, 1], F32, tag="dssb")
        nc.scalar.activation(ds_sb[:], ds_ps[:],
                             mybir.ActivationFunctionType.Identity,
                             bias=bd_sb[:NBINS, :], scale=1.0)
        for tchunk in range(TTR_LEN // 512):
            sl = slice(512 * tchunk, 512 * tchunk + 512)
            oh_sb = stiny.tile([NBINS, 512], F32, tag="ohchunk")
            nc.sync.dma_start(out=oh_sb[:], in_=t["onehot_o"][:, sl])
            mi_sb = stiny.tile([1, 512], F32, tag="michunk")
            nc.sync.dma_start(out=mi_sb[:], in_=t["minf_row"][:, sl])
            ttr_ps = ptiny.tile([1, 512], F32, tag="ttrps")
            nc.tensor.matmul(ttr_ps[:], lhsT=ds_sb[:], rhs=oh_sb[:],
                             start=True, stop=True)
            ttr_sb = stiny.tile([1, 512], F32, tag="ttrchunk")
            nc.vector.tensor_add(ttr_sb[:], ttr_ps[:], mi_sb[:])
            nc.sync.dma_start(
                out=t["ttr_dram"].ap().rearrange(
                    "(a b) o -> a (b o)", a=1
                )[:, sl],
                in_=ttr_sb[:],
            )

    # ---------- width scorer ws[0:30] ----------
    with tc.tile_pool(name="wsps", bufs=2, space="PSUM") as pws, \
         tc.tile_pool(name="wssb", bufs=1) as sws:
        hidw = sws.tile([P, 8, MAXW], F32, tag="hidw")
        for ht in range(8):
            hw_ps = pws.tile([P, MAXW], F32, tag="hwps")
            nc.tensor.matmul(hw_ps[:], lhsT=ww0_sb[:, 128 * ht:128 * ht + 128],
                             rhs=wsembT_sb[:], start=True, stop=True)
            nc.scalar.activation(hidw[:, ht, :], hw_ps[:],
                                 mybir.ActivationFunctionType.Relu,
                                 bias=bm0_like(bw0_sb, ht), scale=1.0)
        ws_ps = pws.tile([MAXW, 1], F32, tag="wsps")
        for ht in range(8):
            nc.tensor.matmul(ws_ps[:], lhsT=hidw[:, ht, :],
                             rhs=ww1_sb[:, ht:ht + 1],
                             start=(ht == 0), stop=(ht == 7))
        ws_sb = sws.tile([MAXW, 1], F32, tag="wssb")
        nc.scalar.activation(ws_sb[:], ws_ps[:],
                             mybir.ActivationFunctionType.Identity,
                             bias=bw1_sb[:MAXW, :], scale=1.0)
        nc.sync.dma_start(out=t["ws_dram"][:, :], in_=ws_sb[:])

    # ---------- word-attention exp(wa) for my L-shard ----------
    if stages < 2:
        ctx_main.__exit__(None, None, None)
        ctx_const.__exit__(None, None, None)
        return
    with tc.tile_pool(name="wa_in", bufs=3) as pwain, \
         tc.tile_pool(name="wa_scr", bufs=2) as pwascr, \
         tc.tile_pool(name="wa_ps", bufs=2, space="PSUM") as pwaps:
        watile = main.tile([P, 16], F32, tag="watile")
        for lt in range(16):
            bt = pwain.tile([P, B], F32, tag="bert_t")
            nc.sync.dma_start(out=bt[:],
                              in_=t["bert_slice"][128 * lt:128 * lt + 128, :])
            scr = pwascr.tile([P, B], F32, tag="wascr")
            nc.vector.scalar_tensor_tensor(
                out=scr[:], in0=bt[:], scalar=1.0, in1=wspan_sb[:],
                op0=mybir.AluOpType.mult, op1=mybir.AluOpType.mult,
                accum_out=watile[:, lt:lt + 1],
            )
        ewatile = main.tile([P, 16], F32, tag="ewatile")
        nc.scalar.activation(ewatile[:], watile[:],
                             mybir.ActivationFunctionType.Exp,
                             bias=bspan_sb[:], scale=1.0)
        ewa_ps = pwaps.tile([16, P], F32, tag="ewaps")
        nc.tensor.transpose(ewa_ps[:], ewatile[:], id128_sb[:])
        ewa_sb = main.tile([16, P], F32, tag="ewasb")
        nc.scalar.copy(ewa_sb[:], ewa_ps[:])
        nc.sync.dma_start(
            out=t["ewa_mine"].ap().rearrange("(a b) o -> a (b o)", a=16),
            in_=ewa_sb[:],
        )
    if stages >= 3:
        nc.gpsimd.collective_compute(
            "AllGather", mybir.AluOpType.bypass, replica_groups=groups,
            ins=[t["ewa_mine"].ap().opt()], outs=[t["ewa_gath"].ap().opt()],
        )
    if stages < 4:
        ctx_main.__exit__(None, None, None)
        ctx_const.__exit__(None, None, None)
        return

    # ---------- mention embeddings (transposed, padded) ----------
    # meT_loc[p, k, col] : row d = 128k + p, col = local mention
    # (cols 0:128 = role-A block, 128:256 = role-B block)
    meT_loc = main.tile([P, KT, 2 * P], F32R, tag="meT_loc")
    ms_row = main.tile([1, 2 * P], F32R, tag="ms_row")

    with tc.tile_pool(name="gwin", bufs=2) as pg, \
         tc.tile_pool(name="gmisc", bufs=2) as pgm, \
         tc.tile_pool(name="attn_sc", bufs=2) as pattn, \
         tc.tile_pool(name="head_ps", bufs=2, space="PSUM") as phead, \
         tc.tile_pool(name="tp_ps", bufs=2, space="PSUM") as ptp, \
         tc.tile_pool(name="diag", bufs=2) as pdiag:

        def transpose_to_meT(src_ap, kt0, ntile, colblk, npart=P):
            """PE-transpose [128, ntile*128] f32r src into meT_loc k-tiles."""
            for i in range(ntile):
                ps = ptp.tile([P, P], F32R, tag="tp")
                nc.tensor.transpose(
                    ps[:npart, :], src_ap[:, P * i:P * i + npart],
                    id128r_sb[:]
                )
                nc.scalar.copy(
                    meT_loc[:npart, kt0 + i, colblk * P:colblk * P + P],
                    ps[:npart, :],
                )

        for colblk, role in ((0, "A"), (1, "B")):
            starts = idx["starts" + role]
            widthsf = idx["widthsf" + role]
            widths = idx["widths" + role]
            ends = idx["ends" + role]
            csl = slice(colblk * P, colblk * P + P)

            # window attention weights
            ewin = pattn.tile([P, MAXW + 1], F32, tag="ewin")
            nc.gpsimd.indirect_dma_start(
                out=ewin[:], out_offset=None,
                in_=t["ewa_gath"][:, :],
                in_offset=IndirectOffsetOnAxis(ap=starts[:, :1], axis=0),
            )
            mask = pattn.tile([P, MAXW + 1], F32, tag="mask")
            nc.vector.tensor_scalar(
                mask[:], jrep_sb[:], widthsf[:, :1], None,
                op0=mybir.AluOpType.is_le,
            )
            ewm = pattn.tile([P, MAXW + 1], F32, tag="ewm")
            nc.vector.tensor_mul(ewm[:], ewin[:], mask[:])
            denom = pattn.tile([P, 1], F32, tag="denom")
            nc.vector.tensor_reduce(denom[:], ewm[:],
                                    axis=mybir.AxisListType.X,
                                    op=mybir.AluOpType.add)
            rden = pattn.tile([P, 1], F32, tag="rden")
            nc.vector.reciprocal(rden[:], denom[:])
            attn = pattn.tile([P, MAXW + 1], F32, tag="attn")
            nc.vector.tensor_scalar(attn[:], ewm[:], rden[:, :1], None,
                                    op0=mybir.AluOpType.mult)

            # head embedding: 31 accumulated diag matmuls over 2 window halves
            head_ps = phead.tile([P, B], F32, tag="head")
            ghalf0 = pg.tile([P, 16 * B], F32R, tag="g")
            nc.gpsimd.indirect_dma_start(
                out=ghalf0[:], out_offset=None,
                in_=t["bert_emb"][:, :],
                in_offset=IndirectOffsetOnAxis(ap=starts[:, :1], axis=0),
            )
            for j in range(16):
                dg = pdiag.tile([P, P], F32R, tag="diag")
                nc.vector.tensor_scalar(dg[:], id128_sb[:],
                                        attn[:, j:j + 1], None,
                                        op0=mybir.AluOpType.mult)
                nc.tensor.matmul(head_ps[:, :512], lhsT=dg[:],
                                 rhs=ghalf0[:, B * j:B * j + 512],
                                 start=(j == 0), stop=False)
                nc.tensor.matmul(head_ps[:, 512:], lhsT=dg[:],
                                 rhs=ghalf0[:, B * j + 512:B * j + B],
                                 start=(j == 0), stop=False)
            # start embedding transpose (row j=0 of the window)
            transpose_to_meT(ghalf0[:, 0:B], 0, 6, colblk)

            ghalf1 = pg.tile([P, 16 * B], F32R, tag="g")
            nc.gpsimd.indirect_dma_start(
                out=ghalf1[:], out_offset=None,
                in_=t["bert_emb"][:, :],
                in_offset=IndirectOffsetOnAxis(ap=starts[:, :1], axis=0),
                element_offset=15 * B,
            )
            for j in range(16, MAXW + 1):
                dg = pdiag.tile([P, P], F32R, tag="diag")
                nc.vector.tensor_scalar(dg[:], id128_sb[:],
                                        attn[:, j:j + 1], None,
                                        op0=mybir.AluOpType.mult)
                jl = j - 15
                nc.tensor.matmul(head_ps[:, :512], lhsT=dg[:],
                                 rhs=ghalf1[:, B * jl:B * jl + 512],
                                 start=False, stop=(j == MAXW))
                nc.tensor.matmul(head_ps[:, 512:], lhsT=dg[:],
                                 rhs=ghalf1[:, B * jl + 512:B * jl + B],
                                 start=False, stop=(j == MAXW))
            head_sb = pgm.tile([P, B], F32R, tag="head_sb")
            nc.scalar.copy(head_sb[:], head_ps[:])
            transpose_to_meT(head_sb[:], 13, 6, colblk)

            # end embeddings
            end_sb = pgm.tile([P, B], F32R, tag="end_sb")
            nc.gpsimd.indirect_dma_start(
                out=end_sb[:], out_offset=None,
                in_=t["bert_emb"][:, :],
                in_offset=IndirectOffsetOnAxis(ap=ends[:, :1], axis=0),
            )
            transpose_to_meT(end_sb[:], 6, 6, colblk)

            # width embeddings -> k-tile 12 rows 0:20
            wemb = pattn.tile([P, FEAT], F32R, tag="wemb")
            nc.gpsimd.indirect_dma_start(
                out=wemb[:], out_offset=None,
                in_=t["width_emb"][:, :],
                in_offset=IndirectOffsetOnAxis(ap=widths[:, :1], axis=0),
            )
            nc.vector.tensor_scalar(meT_loc[:, 12, csl], id128_sb[:],
                                    0.0, None, op0=mybir.AluOpType.mult)
            wps = ptp.tile([P, P], F32R, tag="tp")
            nc.tensor.transpose(wps[:FEAT, :], wemb[:, :FEAT], id128r_sb[:])
            nc.scalar.copy(meT_loc[:FEAT, 12, csl], wps[:FEAT, :])

    # ---------- mention-score MLP ----------
    if stages < 5:
        ctx_main.__exit__(None, None, None)
        ctx_const.__exit__(None, None, None)
        return
    hidT = main.tile([P, 8, 2 * P], F32R, tag="hidT")
    with tc.tile_pool(name="mlp_w", bufs=3) as pmw, \
         tc.tile_pool(name="mlp_ps", bufs=1, space="PSUM") as pmps, \
         tc.tile_pool(name="mlp_sc", bufs=2) as pmsc, \
         tc.tile_pool(name="tp2_ps", bufs=2, space="PSUM") as ptp2:
        for hg in range(2):
            ps_h = [pmps.tile([P, 2 * P], F32, tag=f"mlp{i}", name=f"mlp_ps{i}")
                    for i in range(4)]
            for kt in range(KT):
                slab = pmw.tile([P, 512], F32R, tag="wm0slab")
                nc.gpsimd.dma_start(
                    out=slab[:],
                    in_=t["wm0_p"][128 * kt:128 * kt + 128,
                                   512 * hg:512 * hg + 512],
                )
                for hi in range(4):
                    nc.tensor.matmul(
                        ps_h[hi][:],
                        lhsT=slab[:, 128 * hi:128 * hi + 128],
                        rhs=meT_loc[:, kt, :],
                        start=(kt == 0), stop=(kt == KT - 1),
                    )
            for hi in range(4):
                ht = 4 * hg + hi
                nc.scalar.activation(hidT[:, ht, :], ps_h[hi][:],
                                     mybir.ActivationFunctionType.Relu,
                                     bias=bm0_like(bm0_sb, ht), scale=1.0)
        for colblk, role in ((0, "A"), (1, "B")):
            csl = slice(colblk * P, colblk * P + P)
            ms_ps = pmps.tile([P, 1], F32, tag="msps")
            for ht in range(8):
                nc.tensor.matmul(ms_ps[:],
                                 lhsT=hidT[:, ht, csl].bitcast(F32),
                                 rhs=wm1_sb[:, ht:ht + 1],
                                 start=(ht == 0), stop=(ht == 7))
            wsd = pmsc.tile([P, 1], F32, tag="wsd")
            nc.gpsimd.indirect_dma_start(
                out=wsd[:], out_offset=None,
                in_=t["ws_dram"][:, :],
                in_offset=IndirectOffsetOnAxis(
                    ap=idx["widths" + role][:, :1], axis=0
                ),
            )
            ms_col = pmsc.tile([P, 1], F32, tag="mscol")
            nc.vector.scalar_tensor_tensor(
                out=ms_col[:], in0=ms_ps[:], scalar=bm1_sb[:, :1],
                in1=wsd[:], op0=mybir.AluOpType.add, op1=mybir.AluOpType.add,
            )
            msr_ps = ptp2.tile([1, P], F32, tag="msrps")
            nc.tensor.transpose(msr_ps[:], ms_col[:], id128_sb[:])
            nc.scalar.copy(ms_row[:, csl], msr_ps[:])

    # ms/ones rows into meT_loc k-tile 12 (rows 1662/1663)
    ones_row = main.tile([1, 2 * P], F32R, tag="ones_row")
    nc.vector.tensor_scalar(ones_row[:], wspan_sb[0:1, 0:2 * P], 0.0, 1.0,
                            op0=mybir.AluOpType.mult,
                            op1=mybir.AluOpType.add)
    nc.sync.dma_start(out=meT_loc[126:127, 12, :], in_=ms_row[:, :])
    nc.sync.dma_start(out=meT_loc[127:128, 12, :], in_=ones_row[:, :])

    # ---------- share meT across cores ----------
    if stages < 6:
        ctx_main.__exit__(None, None, None)
        ctx_const.__exit__(None, None, None)
        return
    for kt in range(KT):
        nc.sync.dma_start(
            out=t["meT_bounce"][128 * kt:128 * kt + 128, :],
            in_=meT_loc[:, kt, :],
        )
    nc.gpsimd.collective_compute(
        "AllGather", mybir.AluOpType.bypass, replica_groups=groups,
        ins=[t["meT_bounce"].ap().opt()], outs=[t["meT_gath"].ap().opt()],
    )

    # ---------- srcT = W_fast^T-contraction (padded layout) ----------
    if stages < 7:
        ctx_main.__exit__(None, None, None)
        ctx_const.__exit__(None, None, None)
        return
    srcT_loc = main.tile([P, KT, 2 * P], F32R, tag="srcT_loc")
    DO_GROUPS = [(0, 4), (4, 8), (8, 12), (12, 16), (16, 19)]
    with tc.tile_pool(name="src_w", bufs=3) as psw, \
         tc.tile_pool(name="src_ps", bufs=1, space="PSUM") as psps:
        for g0, g1 in DO_GROUPS:
            nd = g1 - g0
            ps_s = [psps.tile([P, 2 * P], F32, tag=f"src{i}", name=f"src_ps{i}")
                    for i in range(nd)]
            for kt in range(KT):
                slab = psw.tile([P, 512], F32R, tag="wfslab")
                nc.gpsimd.dma_start(
                    out=slab[:, :128 * nd],
                    in_=t["w_fast_p"][128 * kt:128 * kt + 128,
                                      128 * g0:128 * g1],
                )
                for di in range(nd):
                    nc.tensor.matmul(
                        ps_s[di][:],
                        lhsT=slab[:, 128 * di:128 * di + 128],
                        rhs=meT_loc[:, kt, :],
                        start=(kt == 0), stop=(kt == KT - 1),
                    )
            for di in range(nd):
                nc.scalar.activation(
                    srcT_loc[:, g0 + di, :], ps_s[di][:],
                    mybir.ActivationFunctionType.Identity,
                    bias=bfast_sb[:, g0 + di:g0 + di + 1], scale=1.0,
                )
    # pair-matmul special rows: lhsT d=1662 -> ones, d=1663 -> ms_i
    nc.sync.dma_start(out=srcT_loc[126:127, 12, :], in_=ones_row[:, :])
    nc.sync.dma_start(out=srcT_loc[127:128, 12, :], in_=ms_row[:, :])

    # ---------- pairwise scores + top-50 ----------
    if stages < 8:
        ctx_main.__exit__(None, None, None)
        ctx_const.__exit__(None, None, None)
        return
    # gathered row block for global j-block g: core min(g,15-g), half 0/1
    fast = {
        "A": main.tile([P, WA_COLS], F32, tag="fastA", name="fastA"),
        "B": main.tile([P, WB_COLS], F32, tag="fastB", name="fastB"),
    }
    with tc.tile_pool(name="pair_rhs", bufs=2) as prhs, \
         tc.tile_pool(name="pair_T", bufs=3) as pT, \
         tc.tile_pool(name="pair_ps", bufs=3, space="PSUM") as pps:
        for chunk in range(NCHUNK_B):
            rhs = prhs.tile([P, KT, 512], F32R, tag="rhs")
            for gi in range(4):
                g = 4 * chunk + gi
                cg = min(g, 15 - g)
                half = 0 if g <= 7 else 1
                src_ap = (
                    t["meT_gath"].ap()
                    .rearrange("(c k p) j -> c p k j", c=NCORES, k=KT, p=P)
                )[cg, :, :, P * half:P * half + P]
                nc.sync.dma_start(
                    out=rhs[:, :, 128 * gi:128 * gi + 128], in_=src_ap
                )
            roles = ["B"] + (["A"] if chunk < NCHUNK_A else [])
            for role in roles:
                colblk = 0 if role == "A" else 1
                ps = pps.tile([P, 512], F32, tag="pairps")
                for kt in range(KT):
                    nc.tensor.matmul(
                        ps[:],
                        lhsT=srcT_loc[:, kt, P * colblk:P * colblk + P],
                        rhs=rhs[:, kt, :],
                        start=(kt == 0), stop=(kt == KT - 1),
                    )
                Ttile = pT.tile([P, 512], F32, tag="T")
                nc.gpsimd.indirect_dma_start(
                    out=Ttile[:], out_offset=None,
                    in_=t["ttr_dram"][:, :],
                    in_offset=IndirectOffsetOnAxis(
                        ap=idx["toff" + role][:, :1], axis=0
                    ),
                    element_offset=512 * chunk,
                )
                nc.vector.tensor_add(
                    fast[role][:, 512 * chunk:512 * chunk + 512],
                    ps[:], Ttile[:],
                )

    # top-50 per row (7 rounds of max8 + match_replace, descending)
    with tc.tile_pool(name="topk", bufs=1) as ptk:
        for role, out_t in (("A", t["outA"]), ("B", t["outB"])):
            out56 = ptk.tile([P, 56], F32, tag="out56" + role)
            fb = fast[role]
            for r in range(7):
                nc.vector.max(out=out56[:, 8 * r:8 * r + 8], in_=fb[:])
                if r < 6:
                    nc.vector.match_replace(
                        out=fb[:], in_to_replace=out56[:, 8 * r:8 * r + 8],
                        in_values=fb[:], imm_value=NEG_SENTINEL,
                    )
            # sentinel values picked up after a row ran dry -> true -inf
            sentmask = ptk.tile([P, 56], mybir.dt.uint32, tag="sentmask" + role,
                                name="sentmask" + role)
            nc.vector.tensor_scalar(sentmask[:], out56[:], -1.0e38, None,
                                    op0=mybir.AluOpType.is_le)
            nc.vector.copy_predicated(
                out56[:], sentmask[:], ninf_sb[:, :1].to_broadcast([P, 56])
            )
            nc.sync.dma_start(out=out_t[:, :], in_=out56[:, :C])

    ctx_main.__exit__(None, None, None)
    ctx_const.__exit__(None, None, None)


def bm0_like(col8, ht):
    return col8[:, ht:ht + 1]


# ---------------------------------------------------------------------------
# host side
# ---------------------------------------------------------------------------

_PROGRAM = None


def _get_program():
    global _PROGRAM
    if _PROGRAM is None:
        _PROGRAM = build_program()
    return _PROGRAM


def _pad_rows(w):
    """[2324, n] -> [2432, n] with the padded-d layout."""
    out = np.zeros((EP, w.shape[1]), np.float32)
    out[:D_WIDTH + FEAT] = w[:D_WIDTH + FEAT]
    out[D_HEAD:] = w[D_WIDTH + FEAT:]
    return out


def _prep_shared(inputs):
    f32 = lambda x: np.ascontiguousarray(np.asarray(x), dtype=np.float32)
    W_fast = f32(inputs["W_fast"])
    b_fast = f32(inputs["b_fast"])
    Wm0 = f32(inputs["Wm0"])

    w_fast_p = _pad_rows(_pad_rows(W_fast).T).T  # pad rows then cols
    bfast_c = _pad_rows(b_fast.reshape(-1, 1))

    wm0_p = np.zeros((EP, 1024), np.float32)
    wm0_p[:, :H] = _pad_rows(Wm0)
    bm0_c = np.zeros((1024, 1), np.float32)
    bm0_c[:H, 0] = f32(inputs["bm0"])
    wm1_c = np.zeros((1024, 1), np.float32)
    wm1_c[:H, 0] = f32(inputs["Wm1"])[:, 0]
    bm1_col = np.full((P, 1), float(np.asarray(inputs["bm1"])[0]), np.float32)

    ww0 = np.zeros((FEAT, 1024), np.float32)
    ww0[:, :H] = f32(inputs["Ww0"])
    bw0_c = np.zeros((1024, 1), np.float32)
    bw0_c[:H, 0] = f32(inputs["bw0"])
    ww1_c = np.zeros((1024, 1), np.float32)
    ww1_c[:H, 0] = f32(inputs["Ww1"])[:, 0]
    bw1_col = np.full((P, 1), float(np.asarray(inputs["bw1"])[0]), np.float32)

    wspan_rep = np.ascontiguousarray(
        np.broadcast_to(f32(inputs["W_span"])[:, 0], (P, B))
    )
    bspan_col = np.full((P, 1), float(np.asarray(inputs["b_span"])[0]),
                        np.float32)

    dsT = np.ascontiguousarray(f32(inputs["dist_scorer_emb"]).T)
    wd = f32(inputs["Wd"])
    bd_col = np.full((P, 1), float(np.asarray(inputs["bd"])[0]), np.float32)

    # reversed distance template one-hot: y < 2047 -> bin(2047 - y)
    bins = np.repeat(np.arange(NBINS), BIN_WIDTHS)  # [65]
    y = np.arange(TTR_LEN)
    k = 2047 - y
    onehot_o = np.zeros((NBINS, TTR_LEN), np.float32)
    pos = k >= 1
    onehot_o[bins[np.minimum(np.where(pos, k, 1), 64)], y] = pos
    minf_row = np.where(pos, 0.0, -np.inf).astype(np.float32)[None, :]

    id128 = np.eye(P, dtype=np.float32)
    jrep = np.ascontiguousarray(
        np.broadcast_to(np.arange(MAXW + 1, dtype=np.float32), (P, MAXW + 1))
    )

    return {
        "bert_emb": f32(inputs["bert_emb"]),
        "w_fast_p": w_fast_p, "bfast_c": bfast_c,
        "wm0_p": wm0_p, "bm0_c": bm0_c, "wm1_c": wm1_c, "bm1_col": bm1_col,
        "ww0": ww0, "bw0_c": bw0_c, "ww1_c": ww1_c, "bw1_col": bw1_col,
        "wsembT": np.ascontiguousarray(f32(inputs["width_scorer_emb"]).T),
        "width_emb": f32(inputs["width_emb"]),
        "wspan_rep": wspan_rep, "bspan_col": bspan_col,
        "dsT": dsT, "wd": wd, "bd_col": bd_col,
        "onehot_o": onehot_o, "minf_row": minf_row,
        "id128": id128, "jrep": jrep,
        "ninf_col": np.full((P, 1), -np.inf, np.float32),
    }


def kernel(**inputs):
    nc = _get_program()
    shared = _prep_shared(inputs)
    starts = np.asarray(inputs["cand_starts"], dtype=np.int32)
    widths = np.asarray(inputs["cand_widths"], dtype=np.int32)
    ends = starts + widths
    bert = shared["bert_emb"]

    i32col = lambda a: np.ascontiguousarray(a.reshape(P, 1), dtype=np.int32)
    f32col = lambda a: np.ascontiguousarray(a.reshape(P, 1), dtype=np.float32)

    in_maps = []
    for c in range(NCORES):
        bA, bB = c, 15 - c
        rA = slice(128 * bA, 128 * bA + 128)
        rB = slice(128 * bB, 128 * bB + 128)
        pm = dict(shared)
        pm["bert_slice"] = np.ascontiguousarray(
            bert[2048 * c:2048 * c + 2048]
        )
        pm["startsA"] = i32col(starts[rA])
        pm["startsB"] = i32col(starts[rB])
        pm["endsA"] = i32col(ends[rA])
        pm["endsB"] = i32col(ends[rB])
        pm["widthsA"] = i32col(widths[rA])
        pm["widthsB"] = i32col(widths[rB])
        pm["widthsfA"] = f32col(widths[rA].astype(np.float32))
        pm["widthsfB"] = f32col(widths[rB].astype(np.float32))
        pm["toffA"] = i32col(2047 - 128 * bA - np.arange(P))
        pm["toffB"] = i32col(2047 - 128 * bB - np.arange(P))
        in_maps.append(pm)

    res = run_bass_kernel_spmd(nc, in_maps, list(range(NCORES)))
    global LAST_RESULTS
    LAST_RESULTS = res
    out = np.zeros((M, C), np.float32)
    for c in range(NCORES):
        out[128 * c:128 * c + 128] = res.results[c]["outA"]
        out[128 * (15 - c):128 * (15 - c) + 128] = res.results[c]["outB"]
    return out


# revision 18
# speedup vs baseline: 414.5960x; 414.5960x over previous
"""Trainium2 Bass kernel for nn_Model_29188597743627 (gnn_message_passing).

Computation (see reference):
  - mention embeddings from bert_emb via start/end gathers, width emb, and
    masked head attention over a <=31-token window per mention
  - mention score MLP + width score
  - pairwise fast antecedent scores fast = src @ me^T + ms_i + ms_j,
    causal mask (j < i) + distance-bucket score, then per-row top-50.

Sharding: 2048 mentions in 16 blocks of 128; core c owns blocks (c, 15-c)
so that causal scan widths are balanced (block k scans j < 128(k+1)).
Every core runs the same SPMD program: block-role A scans a fixed 1024
columns, role B scans 2048 columns; extra columns are killed by the
additive -inf mask template so the structure is core-uniform.

meT (transposed mention embeddings, padded to 2432 rows) is AllGathered
across cores; per-row top-50 is computed with vector.max8/match_replace.
"""

import numpy as np

import concourse.bacc as bacc
import concourse.bass as bass
import concourse.mybir as mybir
import concourse.tile as tile
from concourse.bass import IndirectOffsetOnAxis
from concourse.bass_utils import run_bass_kernel_spmd

P = 128
L = 16384
B = 768
M = 2048
H = 1000
FEAT = 20
MAXW = 30
C = 50
NCORES = 8

# padded mention-embedding layout (d index):
#   [0:768) start | [768:1536) end | [1536:1556) width | pad | 1662 ms/ones |
#   1663 ones/ms | [1664:2432) head
EP = 2432
KT = EP // P  # 19 k-tiles
D_START = 0
D_END = 768
D_WIDTH = 1536
D_HEAD = 1664
E_RAW = 3 * B + FEAT  # 2324

WA_COLS = 1024  # role-A scan width (blocks 0..7 need <=1024)
WB_COLS = 2048  # role-B scan width (blocks 8..15 need <=2048)
NCHUNK_A = WA_COLS // 512  # 2
NCHUNK_B = WB_COLS // 512  # 4

BIN_WIDTHS = [1, 1, 1, 1, 1, 3, 8, 16, 32, 1]
NBINS = len(BIN_WIDTHS)
TTR_LEN = 4096  # reversed template: y<2047 -> ds[bin(2047-y)], else -inf

F32 = mybir.dt.float32
F32R = mybir.dt.float32r
I32 = mybir.dt.int32

NEG_SENTINEL = -3.0e38  # JSON cannot carry -inf in instruction fields


def _r(ap):
    """bitcast an fp32 AP to float32r for full-rate PE matmuls."""
    return ap.bitcast(F32R)


def build_program():
    nc = bacc.Bacc(
        "TRN2", target_bir_lowering=False, debug=False, num_devices=NCORES
    )

    def inp(name, shape, dtype=F32):
        return nc.dram_tensor(name, shape, dtype, kind="ExternalInput")

    # ---- per-core inputs
    bert_emb = inp("bert_emb", [L, B])          # replicated (gather source)
    bert_slice = inp("bert_slice", [M, B])      # rows [2048c, 2048c+2048)
    startsA = inp("startsA", [P, 1], I32)
    startsB = inp("startsB", [P, 1], I32)
    endsA = inp("endsA", [P, 1], I32)
    endsB = inp("endsB", [P, 1], I32)
    widthsA = inp("widthsA", [P, 1], I32)
    widthsB = inp("widthsB", [P, 1], I32)
    widthsfA = inp("widthsfA", [P, 1])
    widthsfB = inp("widthsfB", [P, 1])
    toffA = inp("toffA", [P, 1], I32)           # 2047 - 128*blkA - p
    toffB = inp("toffB", [P, 1], I32)
    # ---- replicated weights / constants
    w_fast_p = inp("w_fast_p", [EP, EP])        # row+col padded W_fast
    bfast_c = inp("bfast_c", [EP, 1])
    wm0_p = inp("wm0_p", [EP, 1024])            # row-padded, col-padded Wm0
    bm0_c = inp("bm0_c", [1024, 1])
    wm1_c = inp("wm1_c", [1024, 1])
    bm1_col = inp("bm1_col", [P, 1])
    ww0 = inp("ww0", [FEAT, 1024])              # col-padded Ww0
    bw0_c = inp("bw0_c", [1024, 1])
    ww1_c = inp("ww1_c", [1024, 1])
    bw1_col = inp("bw1_col", [P, 1])
    wsembT = inp("wsembT", [FEAT, MAXW])        # width_scorer_emb^T
    width_emb = inp("width_emb", [MAXW, FEAT])
    wspan_rep = inp("wspan_rep", [P, B])        # W_span broadcast to 128 rows
    bspan_col = inp("bspan_col", [P, 1])
    dsT = inp("dsT", [FEAT, NBINS])             # dist_scorer_emb^T
    wd = inp("wd", [FEAT, 1])
    bd_col = inp("bd_col", [P, 1])
    onehot_o = inp("onehot_o", [NBINS, TTR_LEN])
    minf_row = inp("minf_row", [1, TTR_LEN])
    id128 = inp("id128", [P, P])
    ninf_col = inp("ninf_col", [P, 1])
    jrep = inp("jrep", [P, MAXW + 1])           # iota 0..30 per row

    outA = nc.dram_tensor("outA", [P, C], F32, kind="ExternalOutput")
    outB = nc.dram_tensor("outB", [P, C], F32, kind="ExternalOutput")

    # ---- internal DRAM
    ttr_dram = nc.dram_tensor("ttr_dram", [TTR_LEN, 1], F32)
    ws_dram = nc.dram_tensor("ws_dram", [MAXW, 1], F32)
    ewa_mine = nc.dram_tensor("ewa_mine", [M, 1], F32)
    ewa_gath = nc.dram_tensor("ewa_gath", [L, 1], F32, addr_space="Shared")
    meT_bounce = nc.dram_tensor("meT_bounce", [EP, 2 * P], F32R)
    meT_gath = nc.dram_tensor(
        "meT_gath", [NCORES * EP, 2 * P], F32R, addr_space="Shared"
    )

    with tile.TileContext(nc) as tc:
        _emit(
            nc, tc,
            bert_emb=bert_emb, bert_slice=bert_slice,
            startsA=startsA, startsB=startsB, endsA=endsA, endsB=endsB,
            widthsA=widthsA, widthsB=widthsB,
            widthsfA=widthsfA, widthsfB=widthsfB,
            toffA=toffA, toffB=toffB,
            w_fast_p=w_fast_p, bfast_c=bfast_c,
            wm0_p=wm0_p, bm0_c=bm0_c, wm1_c=wm1_c, bm1_col=bm1_col,
            ww0=ww0, bw0_c=bw0_c, ww1_c=ww1_c, bw1_col=bw1_col,
            wsembT=wsembT, width_emb=width_emb,
            wspan_rep=wspan_rep, bspan_col=bspan_col,
            dsT=dsT, wd=wd, bd_col=bd_col,
            onehot_o=onehot_o, minf_row=minf_row,
            id128=id128, jrep=jrep, ninf_col=ninf_col,
            outA=outA, outB=outB,
            ttr_dram=ttr_dram, ws_dram=ws_dram,
            ewa_mine=ewa_mine, ewa_gath=ewa_gath,
            meT_bounce=meT_bounce, meT_gath=meT_gath,
        )
    nc.compile()
    return nc


def _emit(nc, tc, **t):
    import os
    stages = int(os.environ.get("KERNEL_STAGES", "8"))
    reps = int(os.environ.get("KERNEL_REPS", "1"))
    groups = [list(range(NCORES))]

    ctx_const = tc.tile_pool(name="const", bufs=1)
    ctx_main = tc.tile_pool(name="main", bufs=1)
    const = ctx_const.__enter__()
    main = ctx_main.__enter__()

    # ---------- small constants into SBUF ----------
    def load_const(name, shape, dtype=F32):
        tl = const.tile(shape, dtype, tag=name)
        nc.sync.dma_start(out=tl[:], in_=t[name][:])
        return tl

    id128_sb = load_const("id128", [P, P])
    id128r_sb = const.tile([P, P], F32R, tag="id128r")
    nc.gpsimd.dma_start(out=id128r_sb[:], in_=t["id128"][:, :])
    jrep_sb = load_const("jrep", [P, MAXW + 1])
    wspan_sb = load_const("wspan_rep", [P, B])
    bspan_sb = load_const("bspan_col", [P, 1])
    dsT_sb = load_const("dsT", [FEAT, NBINS])
    wd_sb = load_const("wd", [FEAT, 1])
    bd_sb = load_const("bd_col", [P, 1])
    wsembT_sb = load_const("wsembT", [FEAT, MAXW])
    ww0_sb = load_const("ww0", [FEAT, 1024])
    bm1_sb = load_const("bm1_col", [P, 1])
    ninf_sb = load_const("ninf_col", [P, 1])
    bw1_sb = load_const("bw1_col", [P, 1])

    # [1024,1]-shaped columns viewed as [128, 8] (col h = rows 128h..)
    def load_cols8(name):
        tl = const.tile([P, 8], F32, tag=name)
        nc.sync.dma_start(
            out=tl[:],
            in_=t[name].ap().rearrange("(h p) o -> p (h o)", p=P),
        )
        return tl

    bm0_sb = load_cols8("bm0_c")
    wm1_sb = load_cols8("wm1_c")
    bw0_sb = load_cols8("bw0_c")
    ww1_sb = load_cols8("ww1_c")
    bfast_sb = const.tile([P, KT], F32, tag="bfast")
    nc.sync.dma_start(
        out=bfast_sb[:],
        in_=t["bfast_c"].ap().rearrange("(k p) o -> p (k o)", p=P),
    )

    idx = {}
    for nm in ("startsA", "startsB", "endsA", "endsB", "widthsA", "widthsB",
               "toffA", "toffB"):
        idx[nm] = load_const(nm, [P, 1], I32)
    for nm in ("widthsfA", "widthsfB"):
        idx[nm] = load_const(nm, [P, 1])

    for _rep in range(reps):
        _emit_body(nc, tc, t, stages, groups, const, main, idx, locals())
    ctx_main.__exit__(None, None, None)
    ctx_const.__exit__(None, None, None)


def _emit_body(nc, tc, t, stages, groups, const, main, idx, env):
    id128_sb = env["id128_sb"]; id128r_sb = env["id128r_sb"]
    jrep_sb = env["jrep_sb"]; wspan_sb = env["wspan_sb"]
    bspan_sb = env["bspan_sb"]; dsT_sb = env["dsT_sb"]; wd_sb = env["wd_sb"]
    bd_sb = env["bd_sb"]; wsembT_sb = env["wsembT_sb"]; ww0_sb = env["ww0_sb"]
    bm1_sb = env["bm1_sb"]; ninf_sb = env["ninf_sb"]; bw1_sb = env["bw1_sb"]
    bm0_sb = env["bm0_sb"]; wm1_sb = env["wm1_sb"]; bw0_sb = env["bw0_sb"]
    ww1_sb = env["ww1_sb"]; bfast_sb = env["bfast_sb"]

    # ---------- distance template ttr ----------
    with tc.tile_pool(name="ttiny", bufs=2, space="PSUM") as ptiny, \
         tc.tile_pool(name="stiny", bufs=2) as stiny:
        ds_ps = ptiny.tile([NBINS, 1], F32, tag="dsps")
        nc.tensor.matmul(ds_ps[:], lhsT=dsT_sb[:], rhs=wd_sb[:],
                         start=True, stop=True)
        ds_sb = stiny.tile([NBINS, 1], F32, tag="dssb")
        nc.scalar.activation(ds_sb[:], ds_ps[:],
                             mybir.ActivationFunctionType.Identity,
                             bias=bd_sb[:NBINS, :], scale=1.0)
        for tchunk in range(TTR_LEN // 512):
            sl = slice(512 * tchunk, 512 * tchunk + 512)
            oh_sb = stiny.tile([NBINS, 512], F32, tag="ohchunk")
            nc.sync.dma_start(out=oh_sb[:], in_=t["onehot_o"][:, sl])
            mi_sb = stiny.tile([1, 512], F32, tag="michunk")
            nc.sync.dma_start(out=mi_sb[:], in_=t["minf_row"][:, sl])
            ttr_ps = ptiny.tile([1, 512], F32, tag="ttrps")
            nc.tensor.matmul(ttr_ps[:], lhsT=ds_sb[:], rhs=oh_sb[:],
                             start=True, stop=True)
            ttr_sb = stiny.tile([1, 512], F32, tag="ttrchunk")
            nc.vector.tensor_add(ttr_sb[:], ttr_ps[:], mi_sb[:])
            nc.sync.dma_start(
                out=t["ttr_dram"].ap().rearrange(
                    "(a b) o -> a (b o)", a=1
                )[:, sl],
                in_=ttr_sb[:],
            )

    # ---------- width scorer ws[0:30] ----------
    with tc.tile_pool(name="wsps", bufs=2, space="PSUM") as pws, \
         tc.tile_pool(name="wssb", bufs=1) as sws:
        hidw = sws.tile([P, 8, MAXW], F32, tag="hidw")
        for ht in range(8):
            hw_ps = pws.tile([P, MAXW], F32, tag="hwps")
            nc.tensor.matmul(hw_ps[:], lhsT=ww0_sb[:, 128 * ht:128 * ht + 128],
                             rhs=wsembT_sb[:], start=True, stop=True)
            nc.scalar.activation(hidw[:, ht, :], hw_ps[:],
                                 mybir.ActivationFunctionType.Relu,
                                 bias=bm0_like(bw0_sb, ht), scale=1.0)
        ws_ps = pws.tile([MAXW, 1], F32, tag="wsps")
        for ht in range(8):
            nc.tensor.matmul(ws_ps[:], lhsT=hidw[:, ht, :],
                             rhs=ww1_sb[:, ht:ht + 1],
                             start=(ht == 0), stop=(ht == 7))
        ws_sb = sws.tile([MAXW, 1], F32, tag="wssb")
        nc.scalar.activation(ws_sb[:], ws_ps[:],
                             mybir.ActivationFunctionType.Identity,
                             bias=bw1_sb[:MAXW, :], scale=1.0)
        nc.sync.dma_start(out=t["ws_dram"][:, :], in_=ws_sb[:])

    # ---------- word-attention exp(wa) for my L-shard ----------
    if stages < 2:
        return
    with tc.tile_pool(name="wa_in", bufs=3) as pwain, \
         tc.tile_pool(name="wa_scr", bufs=2) as pwascr, \
         tc.tile_pool(name="wa_ps", bufs=2, space="PSUM") as pwaps:
        watile = main.tile([P, 16], F32, tag="watile")
        for lt in range(16):
            bt = pwain.tile([P, B], F32, tag="bert_t")
            nc.sync.dma_start(out=bt[:],
                              in_=t["bert_slice"][128 * lt:128 * lt + 128, :])
            scr = pwascr.tile([P, B], F32, tag="wascr")
            nc.vector.scalar_tensor_tensor(
                out=scr[:], in0=bt[:], scalar=1.0, in1=wspan_sb[:],
                op0=mybir.AluOpType.mult, op1=mybir.AluOpType.mult,
                accum_out=watile[:, lt:lt + 1],
            )
        ewatile = main.tile([P, 16], F32, tag="ewatile")
        nc.scalar.activation(ewatile[:], watile[:],
                             mybir.ActivationFunctionType.Exp,
                             bias=bspan_sb[:], scale=1.0)
        ewa_ps = pwaps.tile([16, P], F32, tag="ewaps")
        nc.tensor.transpose(ewa_ps[:], ewatile[:], id128_sb[:])
        ewa_sb = main.tile([16, P], F32, tag="ewasb")
        nc.scalar.copy(ewa_sb[:], ewa_ps[:])
        nc.sync.dma_start(
            out=t["ewa_mine"].ap().rearrange("(a b) o -> a (b o)", a=16),
            in_=ewa_sb[:],
        )
    if stages >= 3:
        nc.gpsimd.collective_compute(
            "AllGather", mybir.AluOpType.bypass, replica_groups=groups,
            ins=[t["ewa_mine"].ap().opt()], outs=[t["ewa_gath"].ap().opt()],
        )
    if stages < 4:
        return

    # ---------- mention embeddings (transposed, padded) ----------
    # meT_loc[p, k, col] : row d = 128k + p, col = local mention
    # (cols 0:128 = role-A block, 128:256 = role-B block)
    meT_loc = main.tile([P, KT, 2 * P], F32R, tag="meT_loc")
    ms_row = main.tile([1, 2 * P], F32R, tag="ms_row")

    with tc.tile_pool(name="gwin", bufs=2) as pg, \
         tc.tile_pool(name="gmisc", bufs=2) as pgm, \
         tc.tile_pool(name="attn_sc", bufs=2) as pattn, \
         tc.tile_pool(name="head_ps", bufs=2, space="PSUM") as phead, \
         tc.tile_pool(name="tp_ps", bufs=2, space="PSUM") as ptp, \
         tc.tile_pool(name="diag", bufs=2) as pdiag:

        def transpose_to_meT(src_ap, kt0, ntile, colblk, npart=P):
            """PE-transpose [128, ntile*128] f32r src into meT_loc k-tiles."""
            for i in range(ntile):
                ps = ptp.tile([P, P], F32R, tag="tp")
                nc.tensor.transpose(
                    ps[:npart, :], src_ap[:, P * i:P * i + npart],
                    id128r_sb[:]
                )
                nc.scalar.copy(
                    meT_loc[:npart, kt0 + i, colblk * P:colblk * P + P],
                    ps[:npart, :],
                )

        for colblk, role in ((0, "A"), (1, "B")):
            starts = idx["starts" + role]
            widthsf = idx["widthsf" + role]
            widths = idx["widths" + role]
            ends = idx["ends" + role]
            csl = slice(colblk * P, colblk * P + P)

            # window attention weights
            ewin = pattn.tile([P, MAXW + 1], F32, tag="ewin")
            nc.gpsimd.indirect_dma_start(
                out=ewin[:], out_offset=None,
                in_=t["ewa_gath"][:, :],
                in_offset=IndirectOffsetOnAxis(ap=starts[:, :1], axis=0),
            )
            mask = pattn.tile([P, MAXW + 1], F32, tag="mask")
            nc.vector.tensor_scalar(
                mask[:], jrep_sb[:], widthsf[:, :1], None,
                op0=mybir.AluOpType.is_le,
            )
            ewm = pattn.tile([P, MAXW + 1], F32, tag="ewm")
            nc.vector.tensor_mul(ewm[:], ewin[:], mask[:])
            denom = pattn.tile([P, 1], F32, tag="denom")
            nc.vector.tensor_reduce(denom[:], ewm[:],
                                    axis=mybir.AxisListType.X,
                                    op=mybir.AluOpType.add)
            rden = pattn.tile([P, 1], F32, tag="rden")
            nc.vector.reciprocal(rden[:], denom[:])
            attn = pattn.tile([P, MAXW + 1], F32, tag="attn")
            nc.vector.tensor_scalar(attn[:], ewm[:], rden[:, :1], None,
                                    op0=mybir.AluOpType.mult)

            # head embedding: 31 accumulated diag matmuls over 2 window halves
            head_ps = phead.tile([P, B], F32, tag="head")
            ghalf0 = pg.tile([P, 16 * B], F32R, tag="g")
            nc.gpsimd.indirect_dma_start(
                out=ghalf0[:], out_offset=None,
                in_=t["bert_emb"][:, :],
                in_offset=IndirectOffsetOnAxis(ap=starts[:, :1], axis=0),
            )
            for j in range(16):
                dg = pdiag.tile([P, P], F32R, tag="diag")
                nc.vector.tensor_scalar(dg[:], id128_sb[:],
                                        attn[:, j:j + 1], None,
                                        op0=mybir.AluOpType.mult)
                nc.tensor.matmul(head_ps[:, :512], lhsT=dg[:],
                                 rhs=ghalf0[:, B * j:B * j + 512],
                                 start=(j == 0), stop=False)
                nc.tensor.matmul(head_ps[:, 512:], lhsT=dg[:],
                                 rhs=ghalf0[:, B * j + 512:B * j + B],
                                 start=(j == 0), stop=False)
            # start embedding transpose (row j=0 of the window)
            transpose_to_meT(ghalf0[:, 0:B], 0, 6, colblk)

            ghalf1 = pg.tile([P, 16 * B], F32R, tag="g")
            nc.gpsimd.indirect_dma_start(
                out=ghalf1[:], out_offset=None,
                in_=t["bert_emb"][:, :],
                in_offset=IndirectOffsetOnAxis(ap=starts[:, :1], axis=0),
                element_offset=15 * B,
            )
            for j in range(16, MAXW + 1):
                dg = pdiag.tile([P, P], F32R, tag="diag")
                nc.vector.tensor_scalar(dg[:], id128_sb[:],
                                        attn[:, j:j + 1], None,
                                        op0=mybir.AluOpType.mult)
                jl = j - 15
                nc.tensor.matmul(head_ps[:, :512], lhsT=dg[:],
                                 rhs=ghalf1[:, B * jl:B * jl + 512],
                                 start=False, stop=(j == MAXW))
                nc.tensor.matmul(head_ps[:, 512:], lhsT=dg[:],
                                 rhs=ghalf1[:, B * jl + 512:B * jl + B],
                                 start=False, stop=(j == MAXW))
            head_sb = pgm.tile([P, B], F32R, tag="head_sb")
            nc.scalar.copy(head_sb[:], head_ps[:])
            transpose_to_meT(head_sb[:], 13, 6, colblk)

            # end embeddings
            end_sb = pgm.tile([P, B], F32R, tag="end_sb")
            nc.gpsimd.indirect_dma_start(
                out=end_sb[:], out_offset=None,
                in_=t["bert_emb"][:, :],
                in_offset=IndirectOffsetOnAxis(ap=ends[:, :1], axis=0),
            )
            transpose_to_meT(end_sb[:], 6, 6, colblk)

            # width embeddings -> k-tile 12 rows 0:20
            wemb = pattn.tile([P, FEAT], F32R, tag="wemb")
            nc.gpsimd.indirect_dma_start(
                out=wemb[:], out_offset=None,
                in_=t["width_emb"][:, :],
                in_offset=IndirectOffsetOnAxis(ap=widths[:, :1], axis=0),
            )
            nc.vector.tensor_scalar(meT_loc[:, 12, csl], id128_sb[:],
                                    0.0, None, op0=mybir.AluOpType.mult)
            wps = ptp.tile([P, P], F32R, tag="tp")
            nc.tensor.transpose(wps[:FEAT, :], wemb[:, :FEAT], id128r_sb[:])
            nc.scalar.copy(meT_loc[:FEAT, 12, csl], wps[:FEAT, :])

    # ---------- mention-score MLP ----------
    if stages < 5:
        return
    hidT = main.tile([P, 8, 2 * P], F32R, tag="hidT")
    with tc.tile_pool(name="mlp_w", bufs=3) as pmw, \
         tc.tile_pool(name="mlp_ps", bufs=1, space="PSUM") as pmps, \
         tc.tile_pool(name="mlp_sc", bufs=2) as pmsc, \
         tc.tile_pool(name="tp2_ps", bufs=2, space="PSUM") as ptp2:
        for hg in range(2):
            ps_h = [pmps.tile([P, 2 * P], F32, tag=f"mlp{i}", name=f"mlp_ps{i}")
                    for i in range(4)]
            for kt in range(KT):
                slab = pmw.tile([P, 512], F32R, tag="wm0slab")
                nc.gpsimd.dma_start(
                    out=slab[:],
                    in_=t["wm0_p"][128 * kt:128 * kt + 128,
                                   512 * hg:512 * hg + 512],
                )
                for hi in range(4):
                    nc.tensor.matmul(
                        ps_h[hi][:],
                        lhsT=slab[:, 128 * hi:128 * hi + 128],
                        rhs=meT_loc[:, kt, :],
                        start=(kt == 0), stop=(kt == KT - 1),
                    )
            for hi in range(4):
                ht = 4 * hg + hi
                nc.scalar.activation(hidT[:, ht, :], ps_h[hi][:],
                                     mybir.ActivationFunctionType.Relu,
                                     bias=bm0_like(bm0_sb, ht), scale=1.0)
        for colblk, role in ((0, "A"), (1, "B")):
            csl = slice(colblk * P, colblk * P + P)
            ms_ps = pmps.tile([P, 1], F32, tag="msps")
            for ht in range(8):
                nc.tensor.matmul(ms_ps[:],
                                 lhsT=hidT[:, ht, csl].bitcast(F32),
                                 rhs=wm1_sb[:, ht:ht + 1],
                                 start=(ht == 0), stop=(ht == 7))
            wsd = pmsc.tile([P, 1], F32, tag="wsd")
            nc.gpsimd.indirect_dma_start(
                out=wsd[:], out_offset=None,
                in_=t["ws_dram"][:, :],
                in_offset=IndirectOffsetOnAxis(
                    ap=idx["widths" + role][:, :1], axis=0
                ),
            )
            ms_col = pmsc.tile([P, 1], F32, tag="mscol")
            nc.vector.scalar_tensor_tensor(
                out=ms_col[:], in0=ms_ps[:], scalar=bm1_sb[:, :1],
                in1=wsd[:], op0=mybir.AluOpType.add, op1=mybir.AluOpType.add,
            )
            msr_ps = ptp2.tile([1, P], F32, tag="msrps")
            nc.tensor.transpose(msr_ps[:], ms_col[:], id128_sb[:])
            nc.scalar.copy(ms_row[:, csl], msr_ps[:])

    # ms/ones rows into meT_loc k-tile 12 (rows 1662/1663)
    ones_row = main.tile([1, 2 * P], F32R, tag="ones_row")
    nc.vector.tensor_scalar(ones_row[:], wspan_sb[0:1, 0:2 * P], 0.0, 1.0,
                            op0=mybir.AluOpType.mult,
                            op1=mybir.AluOpType.add)
    nc.sync.dma_start(out=meT_loc[126:127, 12, :], in_=ms_row[:, :])
    nc.sync.dma_start(out=meT_loc[127:128, 12, :], in_=ones_row[:, :])

    # ---------- share meT across cores ----------
    if stages < 6:
        return
    for kt in range(KT):
        nc.sync.dma_start(
            out=t["meT_bounce"][128 * kt:128 * kt + 128, :],
            in_=meT_loc[:, kt, :],
        )
    nc.gpsimd.collective_compute(
        "AllGather", mybir.AluOpType.bypass, replica_groups=groups,
        ins=[t["meT_bounce"].ap().opt()], outs=[t["meT_gath"].ap().opt()],
    )

    # ---------- srcT = W_fast^T-contraction (padded layout) ----------
    if stages < 7:
        return
    srcT_loc = main.tile([P, KT, 2 * P], F32R, tag="srcT_loc")
    DO_GROUPS = [(0, 4), (4, 8), (8, 12), (12, 16), (16, 19)]
    with tc.tile_pool(name="src_w", bufs=3) as psw, \
         tc.tile_pool(name="src_ps", bufs=1, space="PSUM") as psps:
        for g0, g1 in DO_GROUPS:
            nd = g1 - g0
            ps_s = [psps.tile([P, 2 * P], F32, tag=f"src{i}", name=f"src_ps{i}")
                    for i in range(nd)]
            for kt in range(KT):
                slab = psw.tile([P, 512], F32R, tag="wfslab")
                nc.gpsimd.dma_start(
                    out=slab[:, :128 * nd],
                    in_=t["w_fast_p"][128 * kt:128 * kt + 128,
                                      128 * g0:128 * g1],
                )
                for di in range(nd):
                    nc.tensor.matmul(
                        ps_s[di][:],
                        lhsT=slab[:, 128 * di:128 * di + 128],
                        rhs=meT_loc[:, kt, :],
                        start=(kt == 0), stop=(kt == KT - 1),
                    )
            for di in range(nd):
                nc.scalar.activation(
                    srcT_loc[:, g0 + di, :], ps_s[di][:],
                    mybir.ActivationFunctionType.Identity,
                    bias=bfast_sb[:, g0 + di:g0 + di + 1], scale=1.0,
                )
    # pair-matmul special rows: lhsT d=1662 -> ones, d=1663 -> ms_i
    nc.sync.dma_start(out=srcT_loc[126:127, 12, :], in_=ones_row[:, :])
    nc.sync.dma_start(out=srcT_loc[127:128, 12, :], in_=ms_row[:, :])

    # ---------- pairwise scores + top-50 ----------
    if stages < 8:
        return
    # gathered row block for global j-block g: core min(g,15-g), half 0/1
    fast = {
        "A": main.tile([P, WA_COLS], F32, tag="fastA", name="fastA"),
        "B": main.tile([P, WB_COLS], F32, tag="fastB", name="fastB"),
    }
    with tc.tile_pool(name="pair_rhs", bufs=2) as prhs, \
         tc.tile_pool(name="pair_T", bufs=3) as pT, \
         tc.tile_pool(name="pair_ps", bufs=3, space="PSUM") as pps:
        for chunk in range(NCHUNK_B):
            rhs = prhs.tile([P, KT, 512], F32R, tag="rhs")
            for gi in range(4):
                g = 4 * chunk + gi
                cg = min(g, 15 - g)
                half = 0 if g <= 7 else 1
                src_ap = (
                    t["meT_gath"].ap()
                    .rearrange("(c k p) j -> c p k j", c=NCORES, k=KT, p=P)
                )[cg, :, :, P * half:P * half + P]
                nc.sync.dma_start(
                    out=rhs[:, :, 128 * gi:128 * gi + 128], in_=src_ap
                )
            roles = ["B"] + (["A"] if chunk < NCHUNK_A else [])
            for role in roles:
                colblk = 0 if role == "A" else 1
                ps = pps.tile([P, 512], F32, tag="pairps")
                for kt in range(KT):
                    nc.tensor.matmul(
                        ps[:],
                        lhsT=srcT_loc[:, kt, P * colblk:P * colblk + P],
                        rhs=rhs[:, kt, :],
                        start=(kt == 0), stop=(kt == KT - 1),
                    )
                Ttile = pT.tile([P, 512], F32, tag="T")
                nc.gpsimd.indirect_dma_start(
                    out=Ttile[:], out_offset=None,
                    in_=t["ttr_dram"][:, :],
                    in_offset=IndirectOffsetOnAxis(
                        ap=idx["toff" + role][:, :1], axis=0
                    ),
                    element_offset=512 * chunk,
                )
                nc.vector.tensor_add(
                    fast[role][:, 512 * chunk:512 * chunk + 512],
                    ps[:], Ttile[:],
                )

    # top-50 per row (7 rounds of max8 + match_replace, descending)
    with tc.tile_pool(name="topk", bufs=1) as ptk:
        for role, out_t in (("A", t["outA"]), ("B", t["outB"])):
            out56 = ptk.tile([P, 56], F32, tag="out56" + role)
            fb = fast[role]
            for r in range(7):
                nc.vector.max(out=out56[:, 8 * r:8 * r + 8], in_=fb[:])
                if r < 6:
                    nc.vector.match_replace(
                        out=fb[:], in_to_replace=out56[:, 8 * r:8 * r + 8],
                        in_values=fb[:], imm_value=NEG_SENTINEL,
                    )
            # sentinel values picked up after a row ran dry -> true -inf
            sentmask = ptk.tile([P, 56], mybir.dt.uint32, tag="sentmask" + role,
                                name="sentmask" + role)
            nc.vector.tensor_scalar(sentmask[:], out56[:], -1.0e38, None,
                                    op0=mybir.AluOpType.is_le)
            nc.vector.copy_predicated(
                out56[:], sentmask[:], ninf_sb[:, :1].to_broadcast([P, 56])
            )
            nc.sync.dma_start(out=out_t[:, :], in_=out56[:, :C])


def bm0_like(col8, ht):
    return col8[:, ht:ht + 1]


# ---------------------------------------------------------------------------
# host side
# ---------------------------------------------------------------------------

_PROGRAM = None


def _get_program():
    global _PROGRAM
    if _PROGRAM is None:
        _PROGRAM = build_program()
    return _PROGRAM


def _pad_rows(w):
    """[2324, n] -> [2432, n] with the padded-d layout."""
    out = np.zeros((EP, w.shape[1]), np.float32)
    out[:D_WIDTH + FEAT] = w[:D_WIDTH + FEAT]
    out[D_HEAD:] = w[D_WIDTH + FEAT:]
    return out


def _prep_shared(inputs):
    f32 = lambda x: np.ascontiguousarray(np.asarray(x), dtype=np.float32)
    W_fast = f32(inputs["W_fast"])
    b_fast = f32(inputs["b_fast"])
    Wm0 = f32(inputs["Wm0"])

    w_fast_p = _pad_rows(_pad_rows(W_fast).T).T  # pad rows then cols
    bfast_c = _pad_rows(b_fast.reshape(-1, 1))

    wm0_p = np.zeros((EP, 1024), np.float32)
    wm0_p[:, :H] = _pad_rows(Wm0)
    bm0_c = np.zeros((1024, 1), np.float32)
    bm0_c[:H, 0] = f32(inputs["bm0"])
    wm1_c = np.zeros((1024, 1), np.float32)
    wm1_c[:H, 0] = f32(inputs["Wm1"])[:, 0]
    bm1_col = np.full((P, 1), float(np.asarray(inputs["bm1"])[0]), np.float32)

    ww0 = np.zeros((FEAT, 1024), np.float32)
    ww0[:, :H] = f32(inputs["Ww0"])
    bw0_c = np.zeros((1024, 1), np.float32)
    bw0_c[:H, 0] = f32(inputs["bw0"])
    ww1_c = np.zeros((1024, 1), np.float32)
    ww1_c[:H, 0] = f32(inputs["Ww1"])[:, 0]
    bw1_col = np.full((P, 1), float(np.asarray(inputs["bw1"])[0]), np.float32)

    wspan_rep = np.ascontiguousarray(
        np.broadcast_to(f32(inputs["W_span"])[:, 0], (P, B))
    )
    bspan_col = np.full((P, 1), float(np.asarray(inputs["b_span"])[0]),
                        np.float32)

    dsT = np.ascontiguousarray(f32(inputs["dist_scorer_emb"]).T)
    wd = f32(inputs["Wd"])
    bd_col = np.full((P, 1), float(np.asarray(inputs["bd"])[0]), np.float32)

    # reversed distance template one-hot: y < 2047 -> bin(2047 - y)
    bins = np.repeat(np.arange(NBINS), BIN_WIDTHS)  # [65]
    y = np.arange(TTR_LEN)
    k = 2047 - y
    onehot_o = np.zeros((NBINS, TTR_LEN), np.float32)
    pos = k >= 1
    onehot_o[bins[np.minimum(np.where(pos, k, 1), 64)], y] = pos
    minf_row = np.where(pos, 0.0, -np.inf).astype(np.float32)[None, :]

    id128 = np.eye(P, dtype=np.float32)
    jrep = np.ascontiguousarray(
        np.broadcast_to(np.arange(MAXW + 1, dtype=np.float32), (P, MAXW + 1))
    )

    return {
        "bert_emb": f32(inputs["bert_emb"]),
        "w_fast_p": w_fast_p, "bfast_c": bfast_c,
        "wm0_p": wm0_p, "bm0_c": bm0_c, "wm1_c": wm1_c, "bm1_col": bm1_col,
        "ww0": ww0, "bw0_c": bw0_c, "ww1_c": ww1_c, "bw1_col": bw1_col,
        "wsembT": np.ascontiguousarray(f32(inputs["width_scorer_emb"]).T),
        "width_emb": f32(inputs["width_emb"]),
        "wspan_rep": wspan_rep, "bspan_col": bspan_col,
        "dsT": dsT, "wd": wd, "bd_col": bd_col,
        "onehot_o": onehot_o, "minf_row": minf_row,
        "id128": id128, "jrep": jrep,
        "ninf_col": np.full((P, 1), -np.inf, np.float32),
    }


def kernel(**inputs):
    nc = _get_program()
    shared = _prep_shared(inputs)
    starts = np.asarray(inputs["cand_starts"], dtype=np.int32)
    widths = np.asarray(inputs["cand_widths"], dtype=np.int32)
    ends = starts + widths
    bert = shared["bert_emb"]

    i32col = lambda a: np.ascontiguousarray(a.reshape(P, 1), dtype=np.int32)
    f32col = lambda a: np.ascontiguousarray(a.reshape(P, 1), dtype=np.float32)

    in_maps = []
    for c in range(NCORES):
        bA, bB = c, 15 - c
        rA = slice(128 * bA, 128 * bA + 128)
        rB = slice(128 * bB, 128 * bB + 128)
        pm = dict(shared)
        pm["bert_slice"] = np.ascontiguousarray(
            bert[2048 * c:2048 * c + 2048]
        )
        pm["startsA"] = i32col(starts[rA])
        pm["startsB"] = i32col(starts[rB])
        pm["endsA"] = i32col(ends[rA])
        pm["endsB"] = i32col(ends[rB])
        pm["widthsA"] = i32col(widths[rA])
        pm["widthsB"] = i32col(widths[rB])
        pm["widthsfA"] = f32col(widths[rA].astype(np.float32))
        pm["widthsfB"] = f32col(widths[rB].astype(np.float32))
        pm["toffA"] = i32col(2047 - 128 * bA - np.arange(P))
        pm["toffB"] = i32col(2047 - 128 * bB - np.arange(P))
        in_maps.append(pm)

    res = run_bass_kernel_spmd(nc, in_maps, list(range(NCORES)))
    global LAST_RESULTS
    LAST_RESULTS = res
    out = np.zeros((M, C), np.float32)
    for c in range(NCORES):
        out[128 * c:128 * c + 128] = res.results[c]["outA"]
        out[128 * (15 - c):128 * (15 - c) + 128] = res.results[c]["outB"]
    return out
